# revision 1
# baseline (speedup 1.0000x reference)
"""Trainium2 Bass kernel for nn_AccumulatingModule (histogram_binning).

Problem: out = score_matrix.at[qt, p, ol1, ol2].add(at1*at2) — a scatter-add of
BATCH*PAIR outer-product contributions into a [65, 90, 151, 151] fp32 histogram.

Strategy (8 NeuronCores, SPMD) — delta-only device kernel:
  * The additive delta for each (qt, pair) row is a sum of outer products
    W_j^T @ W_i with W[b,k,:] = attention[b,k] * onehot(label[b,k]).  The
    device computes ONLY these dense deltas from the tiny routed meta input
    (~350 KB/core); score_matrix (533 MB) never touches the device.  The
    host adds deltas into a copy of score_matrix at unshard time (the
    "accumulate deltas" step of the expert-routing recipe).
  * Deltas are emitted as bf16: rel error ~2^-9 of the delta only, on top of
    fp16 W rounding -> ~5e-3 worst-case vs the 2e-2 gate.
  * Shard the (qt, half) space: 65 qts x 2 halves = 130 sections + 6 dummies
    = 17 per core.  Pattern pairs {(i,j): j in 5..9, i != j}; the host box
    permutation (identity / +5 mod 10) maps them onto each section's real
    pairs, keeping the compiled kernel identical across cores (SPMD).
  * Mixed chunking: the first N2=9 slots/core PSUM-accumulate 2 chunks of
    128 batch rows; the rest are single-chunk.  The router sends qts with
    >128 rows to 2-chunk slots (54 of 72 used at seed distribution).
  * Mains: per j, one 128-col weight load (W_j cols 0..127) streams the 9
    i-blocks in i-slot order into bank-aligned PSUM (2-bank + 1-bank tiles)
    so evacuation is one big strided copy per tile.  Tails (o1 128..150):
    ONE packed strided weight load (tails of all 5 j's = 115 cols) streams
    each W_i once — 151 cols per i instead of a half-rate second pass.
  * W built on DVE from an fp16 iota (all-2-byte operands -> DVE 4x mode);
    PSUM evacuation split DVE/ACT; per-section stores go to section-major
    contiguous DRAM blocks, alternating the two HWDGE rings.
"""

import numpy as np

NUM_QT, NUM_OT, PAIR = 65, 151, 90
BOX = 10
OT = NUM_OT
ROWLEN = OT * OT  # 22801
SECP = 45  # pairs per (half) section
NSEC = 17  # sections per core
N2 = 9  # 2-chunk slots per core (rest are 1-chunk)
NCORES = 8
ROWS_PER_SEC = 256  # meta rows per section slot (2-chunk slots use all 256)
PAT_JS = (5, 6, 7, 8, 9)
MAIN_W = SECP * OT  # 6795 = 5 j-blocks * 9 i-slots * 151
TAIL_P = 115  # 5 j's * 23 tail rows
TAIL_W = BOX * OT  # 1510: one 151-col block per i
OTP = 152  # W box pitch: even so 2-byte DVE ops stay 4B-aligned (4x mode)
B_OT = 6  # tail-store batch: sections per out_tail DMA (batch-major layout)


def _slot_runs(j):
    """i-slot-ordered matmul runs for j's 9 i-values.  Slot s holds
    i = s + (s >= j); slots are grouped 3 per PSUM bank (bank = s//3,
    col = (s%3)*OT) so evacuation is one contiguous copy per 453-col bank.
    Returns [(bank, col_off, i0, glen)]."""
    out = []
    for t in range(3):
        run = []  # list of (slot, i)
        for s in range(3 * t, 3 * t + 3):
            i = s + (1 if s >= j else 0)
            if run and i != run[-1][1] + 1:
                out.append((t, (run[0][0] % 3) * OT, run[0][1], len(run)))
                run = []
            run.append((s, i))
        out.append((t, (run[0][0] % 3) * OT, run[0][1], len(run)))
    return out


MAIN_PLACE = {j: _slot_runs(j) for j in PAT_JS}
# tails: two phases; each phase = one [128, 2, 512] PSUM tile, i-slots 3/bank
TAIL_PHASES = (((0, 1, 2), (3, 4, 5)), ((6, 7, 8), (9,)))


def _chunks_of(sl):
    return 2 if sl < N2 else 1


def build_nc(
    nsec=NSEC,
    internal_io=False,
    null_body=False,
    loop_reps=1,
    no_mm=False,
    no_dma_out=False,
    dma_only=False,
    no_evac=False,
    w_only=False,
    dma_mode=None,
):
    """internal_io=True builds a timing variant: out buffers are Internal
    DRAM (no host transfer), with a tiny external anchor output.
    null_body=True additionally skips the whole section loop.
    loop_reps>1 wraps the body in a hardware For_i loop (timing only).
    Attribution variants: no_mm (skip PE+W, copies from zeros), no_dma_out,
    dma_only (+dma_mode: main_only/read/b2/b4), no_evac (PE+W only),
    w_only (W build only)."""
    import concourse.bacc as bacc
    import concourse.tile as tile
    from concourse import mybir
    from contextlib import ExitStack
    import contextlib

    f32 = mybir.dt.float32
    f16 = mybir.dt.float16  # W dtype: one-hot exact, attention rounded once
    bf16 = mybir.dt.bfloat16  # delta transport dtype

    nc = bacc.Bacc(None, target_bir_lowering=False)
    io_out = {} if internal_io else {"kind": "ExternalOutput"}
    meta = nc.dram_tensor(
        "meta", [nsec * ROWS_PER_SEC, 2 * BOX], f32, kind="ExternalInput"
    )
    iota = nc.dram_tensor("iota", [128, OTP], f16, kind="ExternalInput")
    rdbuf = (
        nc.dram_tensor("rdbuf", [nsec * 128, MAIN_W], bf16)
        if dma_mode == "read"
        else None
    )
    out_main = nc.dram_tensor("out_main", [nsec * 128, MAIN_W], bf16, **io_out)
    # batch-major: batch bb of B sections -> rows bb*115*B + p*B + b, so each
    # partition's B*1510 elements are one contiguous DRAM chunk per DMA.
    out_tail = nc.dram_tensor("out_tail", [nsec * TAIL_P, TAIL_W], bf16, **io_out)
    tail_batches = []
    s0 = 0
    while s0 < nsec:
        tail_batches.append((s0, min(B_OT, nsec - s0)))
        s0 += B_OT
    anchor = (
        nc.dram_tensor("anchor", [128, OT], f16, kind="ExternalOutput")
        if internal_io
        else None
    )

    with tile.TileContext(nc) as tc, ExitStack() as ctx:
        const_pool = ctx.enter_context(tc.tile_pool(name="const", bufs=1))
        meta_pool = ctx.enter_context(tc.tile_pool(name="meta", bufs=4))
        w_pool = ctx.enter_context(tc.tile_pool(name="w", bufs=3))
        om_pool = ctx.enter_context(tc.tile_pool(name="om", bufs=3))
        ot_pool = ctx.enter_context(tc.tile_pool(name="ot", bufs=3))
        pa_pool = ctx.enter_context(tc.tile_pool(name="pa", bufs=2, space="PSUM"))
        pb_pool = ctx.enter_context(tc.tile_pool(name="pb", bufs=2, space="PSUM"))
        pt_pool = ctx.enter_context(tc.tile_pool(name="pt", bufs=1, space="PSUM"))

        iota_t = const_pool.tile([128, OTP], f16)
        nc.sync.dma_start(iota_t[:], iota[:])
        if anchor is not None:
            nc.sync.dma_start(anchor[:, 0:OT], iota_t[:, 0:OT])
        if no_mm or dma_only:
            zmain = const_pool.tile([128, 4 * MAIN_W], bf16)
            nc.vector.memset(zmain[:], 0.0)
            ztail = const_pool.tile([128, 2 * 906], bf16)
            nc.vector.memset(ztail[:], 0.0)

        meta_r = meta.rearrange("(s c r) k -> r s c k", c=2, r=128)

        _otb_cache = [None]
        loop_ctx = tc.For_i(0, loop_reps, 1) if loop_reps > 1 else contextlib.nullcontext()
        with loop_ctx:
          if not (null_body or dma_only):
            mta = meta_pool.tile([128, nsec, 2, 2 * BOX], f32, tag="mta")
            nc.sync.dma_start(mta[:], meta_r)
          for s in range(0 if null_body else nsec):
            nch = _chunks_of(s)
            om_dma, ot_dma = (
                (nc.scalar, nc.sync) if s % 2 == 0 else (nc.sync, nc.scalar)
            )
            if dma_only:
                if dma_mode == "read":
                    rt = om_pool.tile([128, MAIN_W], bf16, tag="om")
                    om_dma.dma_start(rt[:], rdbuf[s * 128 : (s + 1) * 128, :])
                elif dma_mode == "b2" or dma_mode == "b4":
                    B = 2 if dma_mode == "b2" else 4
                    if s % B == 0 and s + B <= nsec:
                        dst = out_main[s * 128 : (s + B) * 128, :].rearrange(
                            "(b p) w -> p b w", b=B
                        )
                        om_dma.dma_start(dst, zmain[:, 0 : B * MAIN_W])
                    elif s + B > nsec and (s % B == 0 or s == (nsec // B) * B):
                        om_dma.dma_start(
                            out_main[s * 128 : (s + 1) * 128, :],
                            zmain[:, 0:MAIN_W],
                        )
                else:
                    om_dma.dma_start(
                        out_main[s * 128 : (s + 1) * 128, :], zmain[:, 0:MAIN_W]
                    )
                    if dma_mode != "main_only":
                        ot_dma.dma_start(
                            out_tail[s * TAIL_P : (s + 1) * TAIL_P, :],
                            ztail[0:TAIL_P, 0:TAIL_W],
                        )
                continue

            # ---- W build on DVE: [128, nch, BOX, OTP] fp16 (4x mode) ----
            w = w_pool.tile([128, 2, BOX, OTP], f16, tag="w")
            wt = w_pool.tile([128, 2, 116], f16, tag="wt")
            if not no_mm:
                for c in range(nch):
                    for k in range(BOX):
                        nc.vector.tensor_scalar(
                            w[:, c, k, :],
                            iota_t[:],
                            mta[:, s, c, k : k + 1],
                            mta[:, s, c, BOX + k : BOX + k + 1],
                            mybir.AluOpType.is_equal,
                            mybir.AluOpType.mult,
                        )
                    nc.vector.tensor_copy(
                        wt[:, c, 0:TAIL_P],
                        w[:, c, 5:BOX, 128:OT],
                    )
            if w_only:
                continue

            om = om_pool.tile([128, MAIN_W], bf16, tag="om")
            s0b = (s // B_OT) * B_OT
            bn = min(B_OT, nsec - s0b)
            if s == s0b:
                otb = ot_pool.tile([128, B_OT, 2 * 906], bf16, tag="ot")
                _otb_cache[0] = otb
            otb = _otb_cache[0]
            sb = s - s0b

            def tail_lhs(c):
                return wt[:, c, 0:TAIL_P]

            # ---- interleaved: tail phase, then mains (tails first so their
            # single-buffered psum tile frees early) ----
            for ph, (kind, arg) in enumerate(
                [("tail", 0), ("main", 5), ("main", 6), ("tail", 1),
                 ("main", 7), ("main", 8), ("main", 9)]
            ):
                if kind == "tail":
                    phase = TAIL_PHASES[arg]
                    obase = arg * 906
                    if no_mm:
                        nc.scalar.copy(
                            otb[0:TAIL_P, sb, obase : obase + 906],
                            ztail[0:TAIL_P, obase : obase + 906],
                        )
                        continue
                    ptt = pt_pool.tile([128, 2, 512], f32, tag="pt")
                    for c in range(nch):
                        for b, slots in enumerate(phase):
                            for si, i in enumerate(slots):
                                # start=True clears has_written for the WHOLE
                                # bank: set it only on the bank's first matmul;
                                # later regions overwrite-on-unset.
                                nc.tensor.matmul(
                                    ptt[0:TAIL_P, b, si * OT : (si + 1) * OT],
                                    tail_lhs(c),
                                    w[:, c, i, 0:OT],
                                    start=(c == 0 and si == 0),
                                    stop=(c == nch - 1),
                                    skip_group_check=True,
                                )
                    if arg == 0:
                        nc.vector.tensor_copy(
                            otb[0:TAIL_P, sb, obase : obase + 906],
                            ptt[0:TAIL_P, :, 0:453],
                        )
                    else:
                        nc.scalar.copy(
                            otb[0:TAIL_P, sb, obase : obase + 906],
                            ptt[0:TAIL_P, :, 0:453],
                        )
                else:
                    j = arg
                    jb = (j - 5) * 9 * OT
                    if no_mm:
                        nc.vector.tensor_copy(
                            om[:, jb : jb + 9 * OT], zmain[:, jb : jb + 9 * OT]
                        )
                        continue
                    psa = pa_pool.tile([128, 2, 512], f32, tag="pa")
                    psb = pb_pool.tile([128, 512], f32, tag="pb")
                    for c in range(nch):
                        seen = set()
                        for b, coff, i0, glen in MAIN_PLACE[j]:
                            dst = (
                                psa[:, b, coff : coff + glen * OT]
                                if b < 2
                                else psb[:, coff : coff + glen * OT]
                            )
                            nc.tensor.matmul(
                                dst,
                                w[:, c, j, 0:128],
                                w[:, c, i0 : i0 + glen, 0:OT],
                                start=(c == 0 and b not in seen),
                                stop=(c == nch - 1),
                                skip_group_check=True,
                            )
                            seen.add(b)
                    if no_evac:
                        continue
                    # slots 0..5 from the 2-bank tile on ACT, 6..8 on DVE
                    nc.scalar.copy(om[:, jb : jb + 906], psa[:, :, 0:453])
                    nc.vector.tensor_copy(
                        om[:, jb + 906 : jb + 1359], psb[:, 0:453]
                    )

            if not (no_dma_out or no_evac):
                om_dma.dma_start(out_main[s * 128 : (s + 1) * 128, :], om[:])
                if s == s0b + bn - 1:
                    dst = out_tail[
                        s0b * TAIL_P : s0b * TAIL_P + TAIL_P * bn, :
                    ].rearrange("(p b) w -> p b w", b=bn)
                    ot_dma.dma_start(dst, otb[0:TAIL_P, 0:bn, 0:TAIL_W])
    return nc


# ---------------------------------------------------------------------------
# host-side routing
# ---------------------------------------------------------------------------


def _route(obj_label, qus_type, attention):
    """Returns (in_maps, placement) where placement[core][slot] =
    (q, h) or None."""
    order = np.argsort(qus_type, kind="stable")
    counts = np.bincount(qus_type, minlength=NUM_QT)
    starts = np.concatenate([[0], np.cumsum(counts)])

    big_qs = [q for q in range(NUM_QT) if counts[q] > 128]
    small_qs = [q for q in range(NUM_QT) if counts[q] <= 128]
    assert counts.max() <= ROWS_PER_SEC, f"qt group of {counts.max()} rows"
    big = [(q, h) for q in big_qs for h in (0, 1)]
    small = [(q, h) for q in small_qs for h in (0, 1)]
    assert len(big) <= NCORES * N2, (
        f"{len(big)} two-chunk sections exceed capacity {NCORES * N2}"
    )
    # fill 2-chunk slots with big sections (round-robin over cores), then
    # spill small sections into leftover 2-chunk slots, then 1-chunk slots.
    placement = [[None] * NSEC for _ in range(NCORES)]
    slots2 = [(c, sl) for sl in range(N2) for c in range(NCORES)]
    slots1 = [(c, sl) for sl in range(N2, NSEC) for c in range(NCORES)]
    pool = big + small
    for (c, sl), sec in zip(slots2 + slots1, pool + [None] * 99):
        placement[c][sl] = sec

    iota_arr = np.full((128, OTP), -1.0, np.float16)
    iota_arr[:, 0:OT] = np.arange(OT, dtype=np.float16)[None, :]
    in_maps = []
    for core in range(NCORES):
        meta = np.zeros((NSEC * ROWS_PER_SEC, 2 * BOX), np.float32)
        for sl in range(NSEC):
            sec = placement[core][sl]
            if sec is None:
                continue
            q, h = sec
            perm = np.array([(x + 5) % 10 if h else x for x in range(BOX)])
            rows = order[starts[q] : starts[q + 1]]
            B = len(rows)
            assert B <= 128 * _chunks_of(sl)
            meta[sl * ROWS_PER_SEC : sl * ROWS_PER_SEC + B, 0:BOX] = obj_label[rows][
                :, perm
            ].astype(np.float32)
            meta[sl * ROWS_PER_SEC : sl * ROWS_PER_SEC + B, BOX:] = attention[rows][
                :, perm
            ]
        in_maps.append({"meta": meta, "iota": iota_arr})
    return in_maps, placement


def _assemble(results, placement, score_matrix):
    """results: per-core dicts with out_main [NSEC*128, MAIN_W] bf16 and
    out_tail [NSEC*115, TAIL_W] bf16.  Returns score + delta, fp32."""
    out2d = np.ascontiguousarray(score_matrix, np.float32).reshape(
        NUM_QT * PAIR, ROWLEN
    ).copy()
    for core in range(NCORES):
        om = np.asarray(results[core]["out_main"], np.float32)
        otl = np.asarray(results[core]["out_tail"], np.float32)
        for sl in range(NSEC):
            sec = placement[core][sl]
            if sec is None:
                continue
            q, h = sec
            perm = np.array([(x + 5) % 10 if h else x for x in range(BOX)])
            dm = om[sl * 128 : (sl + 1) * 128].reshape(128, 5, 9, OT)
            s0b = (sl // B_OT) * B_OT
            bn = min(B_OT, NSEC - s0b)
            b = sl - s0b
            dt = otl[s0b * TAIL_P + b : s0b * TAIL_P + TAIL_P * bn : bn].reshape(
                5, 23, BOX, OT
            )
            rows = np.empty(SECP, np.int64)
            delta = np.empty((SECP, OT, OT), np.float32)
            t = 0
            for jb in range(5):
                j = 5 + jb
                for sI in range(9):
                    i = sI if sI < j else sI + 1
                    I, J = perm[i], perm[j]
                    p = 9 * I + (J if J < I else J - 1)
                    rows[t] = q * PAIR + p
                    delta[t, 0:128, :] = dm[:, jb, sI, :]
                    delta[t, 128:OT, :] = dt[jb, :, i, :]
                    t += 1
            out2d[rows] += delta.reshape(SECP, ROWLEN)
    return out2d.reshape(NUM_QT, PAIR, OT, OT)


_NC_CACHE = {}


def _get_nc(nsec):
    if nsec not in _NC_CACHE:
        nc = build_nc(nsec)
        nc.compile()
        _NC_CACHE[nsec] = nc
    return _NC_CACHE[nsec]


def kernel(obj_label, qus_type, attention, score_matrix):
    from concourse.bass_utils import run_bass_kernel_spmd

    obj_label = np.asarray(obj_label)
    qus_type = np.asarray(qus_type)
    attention = np.asarray(attention, np.float32)
    score_matrix = np.asarray(score_matrix, np.float32)

    in_maps, placement = _route(obj_label, qus_type, attention)
    nc = _get_nc(NSEC)
    res = run_bass_kernel_spmd(nc, in_maps, core_ids=list(range(NCORES)))
    return _assemble(
        [res.results[c] for c in range(NCORES)], placement, score_matrix
    )

